# revision 42
# baseline (speedup 1.0000x reference)
"""Trainium2 kernel for nn_KernelEncodingLayer (von Mises kernel encoding).

Math
----
reference computes, per key n and bin b:
    logits[n,b] = sum_f mag[n,f] * sum_k w[b,f,k] * exp(kappa*(cos(angle[n,f]-mu_eff[b,f,k])-1))

The von Mises kernel expands exactly in a Fourier series (Bessel coefficients):
    exp(kappa*cos(d))*exp(-kappa) = e^-kappa * [I_0(kappa) + 2*sum_m I_m(kappa) cos(m d)]
kappa <= 1 so the series converges superexponentially; truncating cos at m<=2
and sin at m<=3 leaves ~7e-3 max relative error (gate is 2e-2).

With r = mag, u = cos(angle) = x/r, y = r*sin(angle), the needed features are
p_j = r*u^j and q_j = y*u^j, folded with host-side Chebyshev/Bessel math into
per-(bin,freq) weights.  Device chunk layout (contraction rows = 128
partitions; top 64 = p-feature per freq, bottom 64 = q-feature per freq):
    chunk0 = [x ; y  ]  -> (P1, Q0)
    chunk1 = [xu; yu ]  -> (P2, Q1)
    chunk2 = [r ; yu2]  -> (P0, Q2)

Device kernel (per core, 1024 keys), everything fp16 on the wire, fp32 PSUM:
host ships XY=[x;y], XX=[x;x], YY=[y;y] so the whole elementwise chain runs as
full-128-partition ops with no cross-partition copies:
    sq=XX*XX (V)   syb=YY*YY (A square)   r2=sq+syb (V)
    rf=sqrt(r2+eps) (A) = [r;r]           ir=1/rf (V)    uf=XX*ir=[u;u] (V)
    W1=XY*uf=[xu;yu] (V)  W2.bot=W1.bot*uf.bot (V)  W2.top=copy rf.top (A)
then a [128 x key-half] @ [128 x 128bins] PE matmul per chunk accumulated in
PSUM (2 banks of 512 keys), evicted fp16 (bias is added on host), DMA'd out.

Sharding: data-parallel over keys across 8 cores; weights replicated.
"""

import math

import numpy as np

import concourse.bacc as bacc
import concourse.bass as bass
import concourse.mybir as mybir
import concourse.tile as tile
from concourse._compat import with_exitstack
from concourse.bass_utils import run_bass_kernel_spmd
from concourse.mybir import AluOpType

# problem shape (hardcoded per harness contract)
NKEYS = 8192
NBINS = 128
NFREQ = 64
NCORES = 8
KPC = NKEYS // NCORES  # 1024 keys per core
NCHUNK = 3  # contraction chunks: cos harmonics m<=2, sin m<=3
NSPLIT = 2  # key blocks per core for pipelining (PSUM bank per block)
BLK = KPC // NSPLIT

F16 = mybir.dt.float16
F32 = mybir.dt.float32
EPS_GUARD = 1e-6  # r2 guard so 1/r stays bounded

AFT = mybir.ActivationFunctionType


# ----------------------------------------------------------------------------
# host-side math: Bessel I_m and Chebyshev coefficient folding
# ----------------------------------------------------------------------------

def _bessel_i(m: int, x: np.ndarray) -> np.ndarray:
    x = np.asarray(x, np.float64)
    s = np.zeros_like(x)
    for j in range(24):
        s = s + (x / 2.0) ** (2 * j + m) / (math.factorial(j) * math.factorial(j + m))
    return s


def _cheb_t(m: int) -> np.ndarray:
    T = [np.array([1.0]), np.array([0.0, 1.0])]
    while len(T) <= m:
        a = np.zeros(len(T[-1]) + 1)
        a[1:] = 2 * T[-1]
        a[: len(T[-2])] -= T[-2]
        T.append(a)
    return T[m]


def _cheb_u(m: int) -> np.ndarray:
    U = [np.array([1.0]), np.array([0.0, 2.0])]
    while len(U) <= m:
        a = np.zeros(len(U[-1]) + 1)
        a[1:] = 2 * U[-1]
        a[: len(U[-2])] -= U[-2]
        U.append(a)
    return U[m]


def _build_device_weights(reference_angles, mu, kappa, weight) -> np.ndarray:
    """Fold per-(bin,freq) coefficients into [128, NCHUNK*NBINS] fp16.

    Column block c holds chunk c's weights; rows 0:64 multiply the p-feature,
    rows 64:128 the q-feature of that chunk.
    """
    mc, ms = 2, 3  # cos harmonics m<=mc, sin m<=ms
    mu_eff = np.asarray(mu, np.float64) + np.asarray(reference_angles, np.float64)[None, :, None]
    kap = np.asarray(kappa, np.float64)
    w = np.asarray(weight, np.float64)

    P = np.zeros((mc + 1, NBINS, NFREQ))  # coeff of p_j = r*u^j
    Q = np.zeros((ms, NBINS, NFREQ))      # coeff of q_j = y*u^j
    for m in range(0, mc + 1):
        eps = 1.0 if m == 0 else 2.0
        coef = w * eps * _bessel_i(m, kap) * np.exp(-kap)
        A = (coef * np.cos(m * mu_eff)).sum(-1)  # (b, f)
        for j, c in enumerate(_cheb_t(m)):
            if c:
                P[j] += c * A
    for m in range(1, ms + 1):
        coef = w * 2.0 * _bessel_i(m, kap) * np.exp(-kap)
        B = (coef * np.sin(m * mu_eff)).sum(-1)
        for j, c in enumerate(_cheb_u(m - 1)):
            if c:
                Q[j] += c * B

    W = np.zeros((128, NCHUNK * NBINS), np.float64)
    pairs = [(P[1], Q[0]), (P[2], Q[1]), (P[0], Q[2])]
    for c, (top, bot) in enumerate(pairs):
        W[:NFREQ, c * NBINS:(c + 1) * NBINS] = top.T  # (f, b)
        W[NFREQ:, c * NBINS:(c + 1) * NBINS] = bot.T
    return np.ascontiguousarray(W.astype(np.float16))


# ----------------------------------------------------------------------------
# device kernel
# ----------------------------------------------------------------------------

NWARM = 6  # dummy matmuls that hold the PE busy (pstate ramp) during fill
WARMF = 256  # free size of each dummy


@with_exitstack
def _device_kernel(ctx, tc: tile.TileContext, out_d, xy_d, xx_d, yy_d, w_d):
    nc = tc.nc
    const = ctx.enter_context(tc.tile_pool(name="const", bufs=1))
    work = ctx.enter_context(tc.tile_pool(name="work", bufs=1))
    psum = ctx.enter_context(tc.tile_pool(name="psum", bufs=1, space="PSUM"))

    # eps doubles as the r2 guard bias for sqrt and as the operand of a tiny
    # warm-up op that pulls the ACT table load into the DMA-fill window
    eps = const.tile([128, 1], F32, tag="eps")
    warm = const.tile([128, 1], F32, tag="warm")
    nc.gpsimd.memset(eps[:], EPS_GUARD)
    nc.scalar.sqrt(warm[:], eps[:])

    xy = const.tile([128, KPC], F16, tag="xy")
    xx = const.tile([128, KPC], F16, tag="xx")
    yy = const.tile([128, KPC], F16, tag="yy")
    wt = const.tile([128, NCHUNK * NBINS], F16, tag="wt")
    # spread issue engines so the transfers start concurrently; xx/yy first
    # (the chain head consumes them), split per half so h0 lands earliest
    nc.sync.dma_start(yy[:, :BLK], yy_d[:, :BLK])
    nc.sync.dma_start(xx[:, :BLK], xx_d[:, :BLK])
    nc.sync.dma_start(xx[:, BLK:], xx_d[:, BLK:])
    nc.sync.dma_start(yy[:, BLK:], yy_d[:, BLK:])
    nc.sync.dma_start(wt[:], w_d[:])
    nc.sync.dma_start(xy[:, :BLK], xy_d[:, :BLK])
    nc.sync.dma_start(xy[:, BLK:], xy_d[:, BLK:])

    sq = work.tile([128, KPC], F16, tag="sq")
    syb = work.tile([128, KPC], F16, tag="syb")
    r2 = work.tile([128, KPC], F16, tag="r2")
    rf32 = work.tile([128, KPC], F32, tag="rf32")
    ir16 = work.tile([128, KPC], F16, tag="ir16")
    xyx = work.tile([128, KPC], F16, tag="xyx")
    uf = work.tile([128, KPC], F16, tag="uf")
    w1 = work.tile([128, KPC], F16, tag="w1")
    w2 = work.tile([128, KPC], F16, tag="w2")
    outt = work.tile([128, KPC], F16, tag="outt")

    HF = NFREQ
    ps = [psum.tile([128, BLK], F32, tag=f"ps{h}", name=f"ps{h}") for h in range(NSPLIT)]

    # PE pstate warm-up: dummy matmuls on a zeroed tile into a scratch bank,
    # keeping the PE continuously busy from t~0 so real matmuls run at full
    # clock.  No data deps -> scheduled during the DMA fill.
    zt = const.tile([128, WARMF], F16, tag="zt")
    nc.vector.memset(zt[:], 0.0)
    psw = psum.tile([128, WARMF], F32, tag="psw")
    for i in range(NWARM):
        nc.tensor.matmul(psw[:], zt[:, 0:128], zt[:], start=True, stop=True)

    def blk(t, h):
        return t[:, h * BLK:(h + 1) * BLK]

    def blkb(t, h):  # bottom half of a block
        return t[HF:, h * BLK:(h + 1) * BLK]

    def blkt(t, h):  # top half of a block
        return t[:HF, h * BLK:(h + 1) * BLK]

    def recip_fast_f16(out, in_):
        # reciprocal_approx_fast with an fp16 output AP: the BITWISE_NOT seed
        # only needs the fp32 INPUT bit layout; the write-side converts.
        from concourse.dve_ops import RECIP_APPROX_FAST_CONSTS, RECIPROCAL_APPROX_FAST
        c = RECIP_APPROX_FAST_CONSTS
        return nc.vector._custom_dve(
            RECIPROCAL_APPROX_FAST, out=out, in0=in_,
            s0=c["s0"], s1=c["s1"], imm2=c["imm2"],
        )

    import contextlib

    with nc.allow_low_precision(reason="fp16 feature chain; validated vs fp64 host sim"):
        for h in range(NSPLIT):
            # h0's chain outranks h1 in the scheduler so engine queues don't
            # stall behind not-yet-ready h1 ops
            with tc.high_priority() if h == 0 else contextlib.nullcontext():
                # chunk0 matmul only needs xy + weights; runs during the chain
                nc.tensor.matmul(ps[h][:], wt[:, 0:NBINS], blk(xy, h), start=True, stop=False)

                # squares: h0's x^2 on V (spine head), h1's on Pool
                if h == 0:
                    nc.vector.tensor_tensor(blk(sq, h), blk(xx, h), blk(xx, h), AluOpType.mult)
                else:
                    nc.gpsimd.tensor_tensor(blk(sq, h), blk(xx, h), blk(xx, h), AluOpType.mult)
                nc.scalar.square(blk(syb, h), blk(yy, h))
                nc.vector.tensor_tensor(blk(r2, h), blk(sq, h), blk(syb, h), AluOpType.add)
                nc.scalar.activation(blk(rf32, h), blk(r2, h), AFT.Sqrt, bias=eps[:])
                recip_fast_f16(blk(ir16, h), blk(rf32, h))
                # xyx = [x^2; xy] is input-only; scheduled into early V gaps.
                # w1 = xyx*(1/r) = [xu; yu] comes straight off the recip.
                nc.vector.tensor_tensor(blk(xyx, h), blk(xy, h), blk(xx, h), AluOpType.mult)
                nc.vector.tensor_tensor(blk(w1, h), blk(xyx, h), blk(ir16, h), AluOpType.mult)
                nc.tensor.matmul(ps[h][:], wt[:, NBINS:2 * NBINS], blk(w1, h), start=False, stop=False)
                # chunk2: top = r as fp16 (a second sqrt, ACT converts), bottom = yu^2
                nc.scalar.activation(blkt(w2, h), blkt(r2, h), AFT.Sqrt, bias=eps[:HF])
                nc.vector.tensor_tensor(blkb(uf, h), blkb(xx, h), blkb(ir16, h), AluOpType.mult)
                nc.vector.tensor_tensor(blkb(w2, h), blkb(w1, h), blkb(uf, h), AluOpType.mult)
                nc.tensor.matmul(ps[h][:], wt[:, 2 * NBINS:3 * NBINS], blk(w2, h), start=False, stop=True)

                # evict PSUM -> SBUF fp16 (bias added on host), then DMA out
                if h % 2 == 0:
                    nc.scalar.copy(blk(outt, h), ps[h][:])
                    nc.scalar.dma_start(out_d[:, h * BLK:(h + 1) * BLK], blk(outt, h))
                else:
                    nc.scalar.copy(blk(outt, h), ps[h][:])
                    nc.scalar.dma_start(out_d[:, h * BLK:(h + 1) * BLK], blk(outt, h))


_COMPILED = None


def _get_compiled():
    global _COMPILED
    if _COMPILED is None:
        nc = bacc.Bacc("TRN2", target_bir_lowering=False, debug=False)
        xy = nc.dram_tensor("xy", [128, KPC], F16, kind="ExternalInput").ap()
        xx = nc.dram_tensor("xx", [128, KPC], F16, kind="ExternalInput").ap()
        yy = nc.dram_tensor("yy", [128, KPC], F16, kind="ExternalInput").ap()
        w = nc.dram_tensor("w", [128, NCHUNK * NBINS], F16, kind="ExternalInput").ap()
        out = nc.dram_tensor("out", [NBINS, KPC], F16, kind="ExternalOutput").ap()
        with tile.TileContext(nc) as tc:
            _device_kernel(tc, out, xy, xx, yy, w)
        nc.compile()
        _COMPILED = nc
    return _COMPILED


# ----------------------------------------------------------------------------
# entry point
# ----------------------------------------------------------------------------

def _run(K, reference_angles, mu, kappa, weight, bias, **spmd_kwargs):
    K = np.ascontiguousarray(np.asarray(K, np.float32))
    x = K[:, 0::2].astype(np.float16)  # (NKEYS, NFREQ) real parts
    y = K[:, 1::2].astype(np.float16)  # imag parts

    W = _build_device_weights(reference_angles, mu, kappa, weight)
    in_maps = []
    for c in range(NCORES):
        sl = slice(c * KPC, (c + 1) * KPC)
        xt = np.ascontiguousarray(x[sl].T)  # (64, KPC)
        yt = np.ascontiguousarray(y[sl].T)
        xy = np.empty((128, KPC), np.float16)
        xy[:NFREQ] = xt
        xy[NFREQ:] = yt
        xx = np.empty((128, KPC), np.float16)
        xx[:NFREQ] = xt
        xx[NFREQ:] = xt
        yyt = np.empty((128, KPC), np.float16)
        yyt[:NFREQ] = yt
        yyt[NFREQ:] = yt
        in_maps.append({"xy": xy, "xx": xx, "yy": yyt, "w": W})

    nc = _get_compiled()
    res = run_bass_kernel_spmd(nc, in_maps, list(range(NCORES)), **spmd_kwargs)

    bias32 = np.asarray(bias, np.float32)
    out = np.empty((NKEYS, NBINS), np.float32)
    for c in range(NCORES):
        out[c * KPC:(c + 1) * KPC] = res.results[c]["out"].T.astype(np.float32)
    out += bias32[None, :]
    return out, res


def kernel(K, reference_angles, mu, kappa, weight, bias):
    out, _ = _run(K, reference_angles, mu, kappa, weight, bias)
    return out
